# revision 78
# baseline (speedup 1.0000x reference)
"""SAGAN-style self-attention on 8 trn2 cores: data-parallel over batch.

Per core (one batch image): x^T [256,4096] bf16 in, out^T [256,4096] f32 out.
  QT/KT = W^T @ xT + b      [32, 4096]   (bias via ACT Identity per-partition)
  V     = x @ Wh            [4096, 256]  ([keys, c] layout; bh folds into bo')
  per 512-query tile, per 128-key-block group of 4:
    T    = KT_blk.T @ QT_tile   [128 keys, 4x512 queries]  (K=32, row-packed)
    expT = exp(T)               ScalarE, PSUM->SBUF, bf16, no max-subtraction
                                (scores ~N(0, 0.58^2), |s| < ~5 -> fp32-safe)
    O'  += V_blk.T @ expT       [256, 512] PSUM accum over 32 key blocks
    es   = e0+e1+e2+e3 (DVE);  Z += ones.T @ es  [1,512] one matmul per group
                                (the Z matmul is emitted one group late so it
                                never stalls the PE on the DVE es chain)
  tail (PE fully decoupled from the Z chain):
    osb  = bf16(O')             frees the O' PSUM right after the last PV
    F    = Wo.T @ osb           one outproj PSUM bank; freed by an f32 copy
    zr   = 1/Z (DVE, 4 chunks); zb = broadcast(zr) via a DRAM round-trip
                                (direct SBUF broadcast re-reads partition 0's
                                bank 128x and starves the PE rhs streams)
    out^T = F * zb + bo'        DVE, deprioritized; bo' = Wo.T bh + bo
  All tail stages are emitted 2-5 group-slots late (software pipeline) so the
  in-order PE queue never head-of-line blocks on DVE/DMA tail work.
"""

import sys

if "/opt/trn_rl_repo" not in sys.path:
    sys.path.insert(0, "/opt/trn_rl_repo")

import ml_dtypes
import numpy as np

import concourse.bass as bass
import concourse.mybir as mybir
import concourse.tile as tile
from concourse.bass_utils import run_bass_kernel_spmd

B, H, W, C = 8, 64, 64, 256
KEY = 32
N = H * W          # 4096 tokens
NT = 512           # query tile (free dim per matmul)
NTILES = N // NT   # 8
MB = 128           # key block (contraction chunk)
NMB = N // MB      # 32
GRP = 4            # key blocks per group: one per PE row strip

BF16 = mybir.dt.bfloat16
F32 = mybir.dt.float32
FT = mybir.ActivationFunctionType


def build_nc() -> bass.Bass:
    nc = bass.Bass()

    zstage = nc.dram_tensor("zstage", [2, 1, NT], F32)
    xT = nc.declare_dram_parameter("xT", [2, 128, N], BF16, isOutput=False)
    wf = nc.declare_dram_parameter("wf", [2, 128, KEY], BF16, isOutput=False)
    wg = nc.declare_dram_parameter("wg", [2, 128, KEY], BF16, isOutput=False)
    wh = nc.declare_dram_parameter("wh", [2, 128, C], BF16, isOutput=False)
    wo = nc.declare_dram_parameter("wo", [2, 128, C], BF16, isOutput=False)
    bfp = nc.declare_dram_parameter("bfp", [KEY, 1], F32, isOutput=False)
    bgp = nc.declare_dram_parameter("bgp", [KEY, 1], F32, isOutput=False)
    bop = nc.declare_dram_parameter("bop", [2, 128, 1], F32, isOutput=False)
    outT = nc.declare_dram_parameter("outT", [2, 128, N], F32, isOutput=True)

    with tile.TileContext(nc) as tc:
        with (
            tc.tile_pool(name="const", bufs=1) as const,
            tc.tile_pool(name="xp", bufs=1) as xp,
            tc.tile_pool(name="vp", bufs=1) as vp,
            tc.tile_pool(name="qk", bufs=1) as qk,
            tc.tile_pool(name="ep", bufs=3) as ep,
            tc.tile_pool(name="esp", bufs=3) as esp,
            tc.tile_pool(name="osb", bufs=2) as osbp,
            tc.tile_pool(name="zp", bufs=2) as zp,
            tc.tile_pool(name="outp", bufs=3) as outp,
            # PSUM budget is 8 banks total (statically reserved per pool/tag):
            #   pt: tag "t"  [128,2048]f32          = 4 banks
            #   po: tags o0/o1 [128,512]f32 (O')    = 2 banks
            #   pf: tag "f" [128,512]f32 (outproj)  = 1 bank
            #   pz: tag "z" [1,512]f32 (softmax Z)  = 1 bank
            # O' is freed by a z-independent bf16 cast right after the last
            # PV matmul, so the next tile's PV never waits on the tail; all
            # tail PE work is emitted 2-4 group-slots late so the PE never
            # waits on the DVE/DMA tail chain.  Projections borrow "t"/"o0".
            tc.tile_pool(name="pt", bufs=1, space="PSUM") as pt,
            tc.tile_pool(name="po", bufs=1, space="PSUM") as po,
            tc.tile_pool(name="pf", bufs=1, space="PSUM") as pf,
            tc.tile_pool(name="pz", bufs=1, space="PSUM") as pz,
        ):
            ones_col = const.tile([128, 1], BF16)   # Z-matmul lhsT
            nc.vector.memset(ones_col, 1.0)
            wf_sb = const.tile([128, 2, KEY], BF16)
            wg_sb = const.tile([128, 2, KEY], BF16)
            wh_sb = const.tile([128, 2, C], BF16)
            wo_sb = const.tile([128, 2, C], BF16)
            bf_sb = const.tile([KEY, 1], F32)
            bg_sb = const.tile([KEY, 1], F32)
            bo_sb = const.tile([128, 2], F32)

            # spread the input DMA launches across two engine queues (a
            # single queue serializes ~650ns of sequencer time per dma_start)
            # and order them so the first Q/K projection starts early: the
            # first two x slices are 512 wide, the rest 1024 (fewer launches)
            XSPANS = [(0, NT), (NT, NT), (2 * NT, 2 * NT),
                      (4 * NT, 2 * NT), (6 * NT, 2 * NT)]
            xts = [
                [xp.tile([128, w], BF16, name=f"xt{cc}_{st}") for st, w in XSPANS]
                for cc in range(2)
            ]
            dq = [nc.sync, nc.gpsimd]
            dqi = 0

            def dma_in(out, in_):
                nonlocal dqi
                dq[dqi % 2].dma_start(out=out, in_=in_)
                dqi += 1

            for cc in range(2):
                dma_in(wf_sb[:, cc, :], wf[cc])
                st, w = XSPANS[0]
                dma_in(xts[cc][0], xT[cc, :, st:st + w])
            dma_in(bf_sb, bfp[:])
            dma_in(bg_sb, bgp[:])
            for cc in range(2):
                dma_in(wg_sb[:, cc, :], wg[cc])
                dma_in(wh_sb[:, cc, :], wh[cc])
            for hi in range(1, len(XSPANS)):
                st, w = XSPANS[hi]
                for cc in range(2):
                    dma_in(xts[cc][hi], xT[cc, :, st:st + w])
            for cc in range(2):
                dma_in(wo_sb[:, cc, :], wo[cc])
                dma_in(bo_sb[:, cc:cc + 1], bop[cc])

            def xs(cc, start, width):
                # column slice of xT chunk cc; never crosses a tile boundary
                for hi, (st, w) in enumerate(XSPANS):
                    if st <= start and start + width <= st + w:
                        return xts[cc][hi][:, start - st: start - st + width]
                raise AssertionError((start, width))

            pp_i = 0

            def proj_psum(shape):
                # alternate between the two borrowed slots for double-buffering
                nonlocal pp_i
                pp_i += 1
                if pp_i % 2:
                    return pt.tile(shape, F32, tag="t", name=f"projps{pp_i}")
                return po.tile(shape, F32, tag="o0", name=f"projps{pp_i}")

            # ---- Q/K projections ----
            # qt_rep [128, N]: Q^T replicated at the four 32-row strip offsets
            # (each row-packed T matmul streams its rhs from its strip's
            # partitions). Strip 0 written by ACT from PSUM, strips 1-3 by
            # SBUF->SBUF DMA.
            # ---- Q/K/V projections, interleaved per x-tile so the PE can
            # start as soon as the first x slice lands and never outruns the
            # input DMA stream ----
            qt_rep = qk.tile([128, N], BF16)
            kt = qk.tile([KEY, N], BF16)
            # kt_stack [128, NMB//4, 128]: strip i of group g holds
            # K^T[:, (4g+i)*128:(4g+i+1)*128] — stationary operands for the
            # 4-way row-packed T matmuls. The regrouping/replication DMAs are
            # emitted per projection tile so they overlap the remaining
            # projection matmuls instead of serializing before attention.
            kt_stack = qk.tile([128, NMB // 4, MB], BF16)
            for g in range(NTILES):
                sl = slice(g * NT, (g + 1) * NT)
                for dst, w_sb, b_sb in ((qt_rep, wf_sb, bf_sb), (kt, wg_sb, bg_sb)):
                    ps = proj_psum([KEY, NT])
                    for cc in range(2):
                        nc.tensor.matmul(
                            ps, w_sb[:, cc, :], xs(cc, g * NT, NT),
                            start=(cc == 0), stop=(cc == 1),
                        )
                    nc.scalar.activation(
                        out=dst[0:KEY, sl], in_=ps, func=FT.Identity, bias=b_sb,
                    )
                for i in range(1, 4):
                    nc.sync.dma_start(
                        out=qt_rep[32 * i:32 * (i + 1), sl], in_=qt_rep[0:KEY, sl]
                    )
                for i in range(4):
                    b = 4 * g + i
                    nc.sync.dma_start(
                        out=kt_stack[32 * i:32 * (i + 1), g, :],
                        in_=kt[:, b * MB:(b + 1) * MB],
                    )

            # ---- V projection -> 32 tiles [128, 256] bf16 ([keys, c]) ----
            v_sb = []
            for mb in range(NMB):
                ps = proj_psum([128, C])
                for cc in range(2):
                    nc.tensor.matmul(
                        ps, xs(cc, mb * MB, MB), wh_sb[:, cc, :],
                        start=(cc == 0), stop=(cc == 1),
                    )
                vt = vp.tile([128, C], BF16, tag=f"v{mb}")
                nc.vector.tensor_copy(out=vt, in_=ps)
                v_sb.append(vt)

            # ---- attention: flat software pipeline over (query-tile, group) ----
            NGRP = NMB // GRP  # 8 groups of 4 key blocks per query tile
            cur = {}            # nt -> (o_ps pair, z_ps)
            tails = {}          # nt -> dict of tail state between stages
            zq = []             # (nt, g, es) whose Z-matmuls are not yet
                                # emitted: 1-slot deferral normally, 2 slots
                                # for a tile's last group (its exp lands late)

            def tail_recip(nt):
                """After the last Z-matmul: copy Z out of PSUM right away
                (frees the single z bank for the next tile without waiting
                on the reciprocals), then 1/Z on DVE (split so it never
                blocks the es chain for long), then broadcast to 128
                partitions via a DRAM round-trip (a direct SBUF broadcast
                DMA re-reads partition 0's bank 128x and starves the PE's
                rhs streams of that bank)."""
                t = tails[nt]
                zr_sb = zp.tile([1, 1, NT], F32, tag="zr", name=f"zr{nt}")
                for i in range(4):
                    qs = slice(i * (NT // 4), (i + 1) * (NT // 4))
                    nc.vector.reciprocal(
                        out=zr_sb[0:1, 0, qs], in_=t["z_ps"][:, qs]
                    )
                st = nt % 2
                nc.sync.dma_start(out=zstage[st], in_=zr_sb[0:1, 0, :])
                zb_sb = zp.tile([128, NT], F32, tag="zb", name=f"zbs{nt}")
                nc.sync.dma_start(
                    out=zb_sb,
                    in_=zstage[st:st + 1].to_broadcast([1, 128, NT]),
                )
                t["zb"] = zb_sb

            def emit_z(nt, g, es):
                """Deferred softmax-denominator matmul (one per group).  The
                last group's Z is deprioritized so the scheduler stops
                hoisting it ahead of the next tile's T-pack/PV run, where it
                would stall the PE on the es chain; the z-bank handoff is
                covered by the early Z copy in tail_recip."""
                z_ps = cur[nt][1] if nt in cur else tails[nt]["z_ps"]
                nc.tensor.matmul(
                    z_ps, ones_col, es, start=(g == 0), stop=(g == NGRP - 1),
                )
                if g == NGRP - 1:
                    tail_recip(nt)

            def emit_tail_f(nt, cp):
                """Outproj half cp (PE), then a z-independent PSUM->SBUF copy
                so the single outproj bank frees without touching the Z
                chain — the PE stream never waits on Z/1/z/broadcast."""
                t = tails[nt]
                csl = slice(cp * 128, (cp + 1) * 128)
                # out^T[c',n] = (sum_c Wo[c,c'] O'[c,n]) / z[n] + bo'[c']
                f_ps = pf.tile([128, NT], F32, tag="f", name=f"f{cp}_{nt}")
                for cc in range(2):
                    nc.tensor.matmul(
                        f_ps, wo_sb[:, cc, csl], t["osb"][cc],
                        start=(cc == 0), stop=(cc == 1),
                    )
                f_sb = outp.tile([128, NT], F32, tag=f"fs{cp}", name=f"fs{cp}_{nt}")
                # ACT copy: keeps the DVE stream clear for the es/Z chain
                # that races the PV stream end at each tile boundary
                nc.scalar.copy(out=f_sb, in_=f_ps)
                t[f"f_sb{cp}"] = f_sb

            def emit_tail_fn(nt, cp):
                """Normalize + bias (DVE) + store for outproj half cp."""
                t = tails[nt]
                nsl = slice(nt * NT, (nt + 1) * NT)
                with tc.high_priority(offset=-300):
                    fn_sb = outp.tile([128, NT], F32, tag="fn", name=f"fn{cp}_{nt}")
                    nc.vector.tensor_mul(fn_sb, t[f"f_sb{cp}"], t["zb"])
                    out_sb = outp.tile([128, NT], F32, tag="out", name=f"out{cp}_{nt}")
                    nc.vector.tensor_scalar_add(out_sb, fn_sb, bo_sb[:, cp:cp + 1])
                    nc.sync.dma_start(out=outT[cp, :, nsl], in_=out_sb)
                if cp == 1:
                    tails.pop(nt)

            def emit_oz(nt, g, e_sb):
                """PV accumulation for group g of tile nt, then the previous
                group's deferred Z-matmul, then this group's DVE block-sums."""
                if g == 0:
                    cur[nt] = (
                        [po.tile([128, NT], F32, tag="o0", name=f"o0_{nt}"),
                         po.tile([128, NT], F32, tag="o1", name=f"o1_{nt}")],
                        pz.tile([1, NT], F32, tag="z", name=f"z{nt}"),
                    )
                o_ps, _ = cur[nt]
                for j in range(GRP):
                    mb = g * GRP + j
                    esl = e_sb[:, j * NT:(j + 1) * NT]
                    first, last = mb == 0, mb == NMB - 1
                    for cc in range(2):
                        nc.tensor.matmul(
                            o_ps[cc],
                            v_sb[mb][:, cc * 128:(cc + 1) * 128],
                            esl,
                            start=first, stop=last,
                        )
                # softmax denominator: DVE tree-sums the 4 key blocks; the
                # ones-matmul itself is deferred one slot (see emit_z)
                e01 = esp.tile([128, NT], BF16, tag="e01", name=f"e01_{nt}_{g}")
                e23 = esp.tile([128, NT], BF16, tag="e23", name=f"e23_{nt}_{g}")
                es = esp.tile([128, NT], BF16, tag="es", name=f"es_{nt}_{g}")
                # the last group's es chain is on the tile's critical path
                # (its exp lands only one slot earlier): let it run ahead of
                # queued tail work in the DVE stream
                with tc.high_priority(offset=60 if g == NGRP - 1 else 0):
                    nc.vector.tensor_add(e01, e_sb[:, 0:NT], e_sb[:, NT:2 * NT])
                    nc.vector.tensor_add(e23, e_sb[:, 2 * NT:3 * NT], e_sb[:, 3 * NT:])
                    nc.vector.tensor_add(es, e01, e23)
                zq.append((nt, g, es))
                while len(zq) > 1:
                    emit_z(*zq.pop(0))
                if g == NGRP - 1:
                    # free O' immediately (z-independent): PSUM -> bf16 SBUF
                    o_ps, z_ps = cur.pop(nt)
                    osb = []
                    for cc in range(2):
                        ot = osbp.tile(
                            [128, NT], BF16, tag=f"os{cc}", name=f"os{cc}_{nt}"
                        )
                        # one cast per engine: the DVE one stays prompt for
                        # the o_ps handoff to the next tile's first PV while
                        # the ACT one keeps the DVE clear for the es chain
                        if cc == 0:
                            nc.vector.tensor_copy(out=ot, in_=o_ps[cc])
                        else:
                            nc.scalar.copy(out=ot, in_=o_ps[cc])
                        osb.append(ot)
                    tails[nt] = {"osb": osb, "z_ps": z_ps}

            # Pipelined one group deep: T-pack(i) ... PV(i-1).  exp(i) on ACT
            # hides under PV(i-1) on PE.  Tail PE work is deferred: Z(i) lands
            # after PV(i+1); outproj halves land 3 and 4 slots after a tile's
            # last PV, so the PE never waits on the DVE/DMA tail chain.
            prev = None
            for nt in range(NTILES):
                nsl = slice(nt * NT, (nt + 1) * NT)
                for g in range(NGRP):
                    # 4-way row-packed score matmuls: strip j contracts its own
                    # 32 rows of the PE array concurrently (measured ~3x).
                    t_ps = pt.tile([128, GRP * NT], F32, tag="t", name=f"t{nt}_{g}")
                    for j in range(GRP):
                        nc.tensor.matmul(
                            t_ps[:, j * NT:(j + 1) * NT],
                            kt_stack[32 * j:32 * (j + 1), g, :],
                            qt_rep[32 * j:32 * (j + 1), nsl],
                            start=True, stop=True,
                            tile_position=(32 * j, 0),
                        )
                    e_sb = ep.tile([128, GRP * NT], BF16, tag="e", name=f"e{nt}_{g}")
                    nc.scalar.activation(out=e_sb, in_=t_ps, func=FT.Exp)
                    if prev is not None:
                        emit_oz(*prev)
                        if prev[0] >= 1:
                            pn = prev[0] - 1
                            if prev[1] == 2:
                                emit_tail_f(pn, 0)
                            elif prev[1] == 3:
                                emit_tail_f(pn, 1)
                            elif prev[1] == 4:
                                emit_tail_fn(pn, 0)
                            elif prev[1] == 5:
                                emit_tail_fn(pn, 1)
                    prev = (nt, g, e_sb)
            emit_oz(*prev)
            while zq:
                emit_z(*zq.pop(0))
            emit_tail_f(NTILES - 1, 0)
            emit_tail_f(NTILES - 1, 1)
            emit_tail_fn(NTILES - 1, 0)
            emit_tail_fn(NTILES - 1, 1)

    _split_multiwaits(nc)
    return nc


def _split_multiwaits(nc: bass.Bass) -> None:
    """This container's walrus accepts at most ONE sync-wait per instruction
    (CoreV3GenImpl setupSyncWait). Tile emits multi-wait instructions; split
    the excess waits onto EventSemaphore carriers inserted just before the
    instruction on the same engine — same-engine program order makes this
    semantics-preserving."""
    import json as _json

    data = _json.loads(mybir.module_to_json_bytes(nc.m))
    uid = 0
    for fn in data["functions"]:
        for bb in fn["blocks"]:
            new = []
            for inst in bb["instructions"]:
                si = inst.get("sync_info")
                waits = (si or {}).get("on_wait") or []
                if len(waits) > 1:
                    for wcmd in waits[:-1]:
                        uid += 1
                        new.append({
                            "debug": inst.get("debug", 0),
                            "engine": inst["engine"],
                            "ins": [], "outs": [],
                            "name": f"syncw-{uid}",
                            "opcode": "EventSemaphore",
                            "sync_info": {"on_update": [], "on_wait": [wcmd]},
                        })
                    si["on_wait"] = [waits[-1]]
                new.append(inst)
            bb["instructions"] = new
    nc.m = mybir.module_from_json_bytes(_json.dumps(data).encode())


_NC = None


def _get_nc():
    global _NC
    if _NC is None:
        _NC = build_nc()
    return _NC


def _prep_maps(x, Wf, bf, Wg, bg, Wh, bh, Wo, bo):
    bft = ml_dtypes.bfloat16
    # V bias folds through the (linear) attention average into the output
    # projection: out = (O'/z) @ Wo + (bh @ Wo + bo)
    bo_prime = (Wo.T.astype(np.float64) @ bh.astype(np.float64)
                + bo.astype(np.float64)).astype(np.float32)
    shared = {
        "wf": np.ascontiguousarray(Wf.reshape(2, 128, KEY).astype(bft)),
        "wg": np.ascontiguousarray(Wg.reshape(2, 128, KEY).astype(bft)),
        "wh": np.ascontiguousarray(Wh.reshape(2, 128, C).astype(bft)),
        "wo": np.ascontiguousarray(Wo.reshape(2, 128, C).astype(bft)),
        "bfp": np.ascontiguousarray(bf.reshape(KEY, 1).astype(np.float32)),
        "bgp": np.ascontiguousarray(bg.reshape(KEY, 1).astype(np.float32)),
        "bop": np.ascontiguousarray(bo_prime.reshape(2, 128, 1)),
    }
    in_maps = []
    for b in range(B):
        xTb = np.ascontiguousarray(
            x[b].reshape(N, C).T.astype(bft).reshape(2, 128, N)
        )
        m = dict(shared)
        m["xT"] = xTb
        in_maps.append(m)
    return in_maps


def run(x, Wf, bf, Wg, bg, Wh, bh, Wo, bo, trace=False, **kw):
    x = np.asarray(x, dtype=np.float32)
    in_maps = _prep_maps(
        x, *(np.asarray(a, dtype=np.float32) for a in (Wf, bf, Wg, bg, Wh, bh, Wo, bo))
    )
    res = run_bass_kernel_spmd(_get_nc(), in_maps, list(range(B)), trace=trace, **kw)
    out = np.empty((B, H, W, C), dtype=np.float32)
    for b in range(B):
        oT = np.asarray(res.results[b]["outT"], dtype=np.float32).reshape(C, N)
        out[b] = oT.T.reshape(H, W, C)
    return out, res


def kernel(x, Wf, bf, Wg, bg, Wh, bh, Wo, bo):
    out, _ = run(x, Wf, bf, Wg, bg, Wh, bh, Wo, bo)
    return out



# revision 79
# speedup vs baseline: 1.0391x; 1.0391x over previous
"""SAGAN-style self-attention on 8 trn2 cores: data-parallel over batch.

Per core (one batch image): x^T [256,4096] bf16 in, out^T [256,4096] f32 out.
  QT/KT = W^T @ xT + b      [32, 4096]   (bias via ACT Identity per-partition)
  V     = x @ Wh            [4096, 256]  ([keys, c] layout; bh folds into bo')
  per 512-query tile, per 128-key-block group of 4:
    T    = KT_blk.T @ QT_tile   [128 keys, 4x512 queries]  (K=32, row-packed)
    expT = exp(T)               ScalarE, PSUM->SBUF, bf16, no max-subtraction
                                (scores ~N(0, 0.58^2), |s| < ~5 -> fp32-safe)
    O'  += V_blk.T @ expT       [256, 512] PSUM accum over 32 key blocks
    es   = e0+e1+e2+e3 (DVE);  Z += ones.T @ es  [1,512] one matmul per group
                                (the Z matmul is emitted one group late so it
                                never stalls the PE on the DVE es chain)
  tail (PE fully decoupled from the Z chain):
    osb  = bf16(O')             frees the O' PSUM right after the last PV
    F    = Wo.T @ osb           one outproj PSUM bank; freed by an f32 copy
    zr   = 1/Z (DVE, 4 chunks); zb = broadcast(zr) via a DRAM round-trip
                                (direct SBUF broadcast re-reads partition 0's
                                bank 128x and starves the PE rhs streams)
    out^T = F * zb + bo'        DVE, deprioritized; bo' = Wo.T bh + bo
  All tail stages are emitted 2-5 group-slots late (software pipeline) so the
  in-order PE queue never head-of-line blocks on DVE/DMA tail work.
"""

import sys

if "/opt/trn_rl_repo" not in sys.path:
    sys.path.insert(0, "/opt/trn_rl_repo")

import ml_dtypes
import numpy as np

import concourse.bass as bass
import concourse.mybir as mybir
import concourse.tile as tile
from concourse.bass_utils import run_bass_kernel_spmd

B, H, W, C = 8, 64, 64, 256
KEY = 32
N = H * W          # 4096 tokens
NT = 512           # query tile (free dim per matmul)
NTILES = N // NT   # 8
MB = 128           # key block (contraction chunk)
NMB = N // MB      # 32
GRP = 4            # key blocks per group: one per PE row strip

BF16 = mybir.dt.bfloat16
F32 = mybir.dt.float32
FT = mybir.ActivationFunctionType


def build_nc() -> bass.Bass:
    nc = bass.Bass()

    zstage = nc.dram_tensor("zstage", [2, 1, NT], F32)
    xT = nc.declare_dram_parameter("xT", [2, 128, N], BF16, isOutput=False)
    wf = nc.declare_dram_parameter("wf", [2, 128, KEY], BF16, isOutput=False)
    wg = nc.declare_dram_parameter("wg", [2, 128, KEY], BF16, isOutput=False)
    wh = nc.declare_dram_parameter("wh", [2, 128, C], BF16, isOutput=False)
    wo = nc.declare_dram_parameter("wo", [2, 128, C], BF16, isOutput=False)
    bfp = nc.declare_dram_parameter("bfp", [KEY, 1], F32, isOutput=False)
    bgp = nc.declare_dram_parameter("bgp", [KEY, 1], F32, isOutput=False)
    bop = nc.declare_dram_parameter("bop", [2, 128, 1], F32, isOutput=False)
    outT = nc.declare_dram_parameter("outT", [2, 128, N], F32, isOutput=True)

    with tile.TileContext(nc) as tc:
        with (
            tc.tile_pool(name="const", bufs=1) as const,
            tc.tile_pool(name="xp", bufs=1) as xp,
            tc.tile_pool(name="vp", bufs=1) as vp,
            tc.tile_pool(name="qk", bufs=1) as qk,
            tc.tile_pool(name="ep", bufs=3) as ep,
            tc.tile_pool(name="esp", bufs=3) as esp,
            tc.tile_pool(name="osb", bufs=2) as osbp,
            tc.tile_pool(name="zp", bufs=2) as zp,
            tc.tile_pool(name="outp", bufs=3) as outp,
            # PSUM budget is 8 banks total (statically reserved per pool/tag):
            #   pt: tag "t"  [128,2048]f32          = 4 banks
            #   po: tags o0/o1 [128,512]f32 (O')    = 2 banks
            #   pf: tag "f" [128,512]f32 (outproj)  = 1 bank
            #   pz: tag "z" [1,512]f32 (softmax Z)  = 1 bank
            # O' is freed by a z-independent bf16 cast right after the last
            # PV matmul, so the next tile's PV never waits on the tail; all
            # tail PE work is emitted 2-4 group-slots late so the PE never
            # waits on the DVE/DMA tail chain.  Projections borrow "t"/"o0".
            tc.tile_pool(name="pt", bufs=1, space="PSUM") as pt,
            tc.tile_pool(name="po", bufs=1, space="PSUM") as po,
            tc.tile_pool(name="pf", bufs=1, space="PSUM") as pf,
            tc.tile_pool(name="pz", bufs=1, space="PSUM") as pz,
        ):
            ones_col = const.tile([128, 1], BF16)   # Z-matmul lhsT
            nc.vector.memset(ones_col, 1.0)
            wf_sb = const.tile([128, 2, KEY], BF16)
            wg_sb = const.tile([128, 2, KEY], BF16)
            wh_sb = const.tile([128, 2, C], BF16)
            wo_sb = const.tile([128, 2, C], BF16)
            bf_sb = const.tile([KEY, 1], F32)
            bg_sb = const.tile([KEY, 1], F32)
            bo_sb = const.tile([128, 2], F32)

            # spread the input DMA launches across two engine queues (a
            # single queue serializes ~650ns of sequencer time per dma_start)
            # and order them so the first Q/K projection starts early: the
            # first two x slices are 512 wide, the rest 1024 (fewer launches)
            XSPANS = [(0, NT), (NT, NT), (2 * NT, 2 * NT),
                      (4 * NT, 2 * NT), (6 * NT, 2 * NT)]
            xts = [
                [xp.tile([128, w], BF16, name=f"xt{cc}_{st}") for st, w in XSPANS]
                for cc in range(2)
            ]
            dq = [nc.sync, nc.gpsimd]
            dqi = 0

            def dma_in(out, in_):
                nonlocal dqi
                dq[dqi % 2].dma_start(out=out, in_=in_)
                dqi += 1

            for cc in range(2):
                dma_in(wf_sb[:, cc, :], wf[cc])
                st, w = XSPANS[0]
                dma_in(xts[cc][0], xT[cc, :, st:st + w])
            dma_in(bf_sb, bfp[:])
            dma_in(bg_sb, bgp[:])
            for cc in range(2):
                dma_in(wg_sb[:, cc, :], wg[cc])
                dma_in(wh_sb[:, cc, :], wh[cc])
            for hi in range(1, len(XSPANS)):
                st, w = XSPANS[hi]
                for cc in range(2):
                    dma_in(xts[cc][hi], xT[cc, :, st:st + w])
            for cc in range(2):
                dma_in(wo_sb[:, cc, :], wo[cc])
                dma_in(bo_sb[:, cc:cc + 1], bop[cc])

            def xs(cc, start, width):
                # column slice of xT chunk cc; never crosses a tile boundary
                for hi, (st, w) in enumerate(XSPANS):
                    if st <= start and start + width <= st + w:
                        return xts[cc][hi][:, start - st: start - st + width]
                raise AssertionError((start, width))

            pp_i = 0

            def proj_psum(shape):
                # alternate between the two borrowed slots for double-buffering
                nonlocal pp_i
                pp_i += 1
                if pp_i % 2:
                    return pt.tile(shape, F32, tag="t", name=f"projps{pp_i}")
                return po.tile(shape, F32, tag="o0", name=f"projps{pp_i}")

            # ---- Q/K projections ----
            # qt_rep [128, N]: Q^T replicated at the four 32-row strip offsets
            # (each row-packed T matmul streams its rhs from its strip's
            # partitions). Strip 0 written by ACT from PSUM, strips 1-3 by
            # SBUF->SBUF DMA.
            # ---- Q/K/V projections, interleaved per x-tile so the PE can
            # start as soon as the first x slice lands and never outruns the
            # input DMA stream ----
            qt_rep = qk.tile([128, N], BF16)
            kt = qk.tile([KEY, N], BF16)
            # kt_stack [128, NMB//4, 128]: strip i of group g holds
            # K^T[:, (4g+i)*128:(4g+i+1)*128] — stationary operands for the
            # 4-way row-packed T matmuls. The regrouping/replication DMAs are
            # emitted per projection tile so they overlap the remaining
            # projection matmuls instead of serializing before attention.
            kt_stack = qk.tile([128, NMB // 4, MB], BF16)
            for g in range(NTILES):
                sl = slice(g * NT, (g + 1) * NT)
                for dst, w_sb, b_sb in ((qt_rep, wf_sb, bf_sb), (kt, wg_sb, bg_sb)):
                    ps = proj_psum([KEY, NT])
                    for cc in range(2):
                        nc.tensor.matmul(
                            ps, w_sb[:, cc, :], xs(cc, g * NT, NT),
                            start=(cc == 0), stop=(cc == 1),
                        )
                    nc.scalar.activation(
                        out=dst[0:KEY, sl], in_=ps, func=FT.Identity, bias=b_sb,
                    )
                for i in range(1, 4):
                    nc.sync.dma_start(
                        out=qt_rep[32 * i:32 * (i + 1), sl], in_=qt_rep[0:KEY, sl]
                    )
                for i in range(4):
                    b = 4 * g + i
                    nc.sync.dma_start(
                        out=kt_stack[32 * i:32 * (i + 1), g, :],
                        in_=kt[:, b * MB:(b + 1) * MB],
                    )

            # ---- V projection -> 32 tiles [128, 256] bf16 ([keys, c]) ----
            v_sb = []
            for mb in range(NMB):
                ps = proj_psum([128, C])
                for cc in range(2):
                    nc.tensor.matmul(
                        ps, xs(cc, mb * MB, MB), wh_sb[:, cc, :],
                        start=(cc == 0), stop=(cc == 1),
                    )
                vt = vp.tile([128, C], BF16, tag=f"v{mb}")
                nc.vector.tensor_copy(out=vt, in_=ps)
                v_sb.append(vt)

            # ---- attention: flat software pipeline over (query-tile, group) ----
            NGRP = NMB // GRP  # 8 groups of 4 key blocks per query tile
            cur = {}            # nt -> (o_ps pair, z_ps)
            tails = {}          # nt -> dict of tail state between stages
            zq = []             # (nt, g, es) whose Z-matmuls are not yet
                                # emitted: 1-slot deferral normally, 2 slots
                                # for a tile's last group (its exp lands late)

            def tail_recip(nt):
                """After the last Z-matmul: copy Z out of PSUM right away
                (frees the single z bank for the next tile without waiting
                on the reciprocals), then 1/Z on DVE (split so it never
                blocks the es chain for long), then broadcast to 128
                partitions via a DRAM round-trip (a direct SBUF broadcast
                DMA re-reads partition 0's bank 128x and starves the PE's
                rhs streams of that bank)."""
                t = tails[nt]
                zr_sb = zp.tile([1, 1, NT], F32, tag="zr", name=f"zr{nt}")
                for i in range(4):
                    qs = slice(i * (NT // 4), (i + 1) * (NT // 4))
                    nc.vector.reciprocal(
                        out=zr_sb[0:1, 0, qs], in_=t["z_ps"][:, qs]
                    )
                st = nt % 2
                nc.sync.dma_start(out=zstage[st], in_=zr_sb[0:1, 0, :])
                zb_sb = zp.tile([128, NT], F32, tag="zb", name=f"zbs{nt}")
                nc.sync.dma_start(
                    out=zb_sb,
                    in_=zstage[st:st + 1].to_broadcast([1, 128, NT]),
                )
                t["zb"] = zb_sb

            def emit_z(nt, g, es):
                """Deferred softmax-denominator matmul (one per group).  The
                last group's Z is deprioritized so the scheduler stops
                hoisting it ahead of the next tile's T-pack/PV run, where it
                would stall the PE on the es chain; the z-bank handoff is
                covered by the early Z copy in tail_recip."""
                z_ps = cur[nt][1] if nt in cur else tails[nt]["z_ps"]
                nc.tensor.matmul(
                    z_ps, ones_col, es, start=(g == 0), stop=(g == NGRP - 1),
                )
                if g == NGRP - 1:
                    tail_recip(nt)

            def emit_tail_f(nt, cp):
                """Outproj half cp (PE), then a z-independent PSUM->SBUF copy
                so the single outproj bank frees without touching the Z
                chain — the PE stream never waits on Z/1/z/broadcast."""
                t = tails[nt]
                csl = slice(cp * 128, (cp + 1) * 128)
                # out^T[c',n] = (sum_c Wo[c,c'] O'[c,n]) / z[n] + bo'[c']
                f_ps = pf.tile([128, NT], F32, tag="f", name=f"f{cp}_{nt}")
                for cc in range(2):
                    nc.tensor.matmul(
                        f_ps, wo_sb[:, cc, csl], t["osb"][cc],
                        start=(cc == 0), stop=(cc == 1),
                    )
                f_sb = outp.tile([128, NT], F32, tag=f"fs{cp}", name=f"fs{cp}_{nt}")
                # ACT copy: keeps the DVE stream clear for the es/Z chain
                # that races the PV stream end at each tile boundary
                nc.scalar.copy(out=f_sb, in_=f_ps)
                t[f"f_sb{cp}"] = f_sb

            def emit_tail_fn(nt, cp):
                """Normalize + bias (DVE) + store for outproj half cp."""
                t = tails[nt]
                nsl = slice(nt * NT, (nt + 1) * NT)
                with tc.high_priority(offset=-300):
                    fn_sb = outp.tile([128, NT], F32, tag="fn", name=f"fn{cp}_{nt}")
                    nc.vector.tensor_mul(fn_sb, t[f"f_sb{cp}"], t["zb"])
                    out_sb = outp.tile([128, NT], F32, tag="out", name=f"out{cp}_{nt}")
                    nc.vector.tensor_scalar_add(out_sb, fn_sb, bo_sb[:, cp:cp + 1])
                    nc.sync.dma_start(out=outT[cp, :, nsl], in_=out_sb)
                if cp == 1:
                    tails.pop(nt)

            def emit_oz(nt, g, e_sb):
                """PV accumulation for group g of tile nt, then the previous
                group's deferred Z-matmul, then this group's DVE block-sums."""
                if g == 0:
                    cur[nt] = (
                        [po.tile([128, NT], F32, tag="o0", name=f"o0_{nt}"),
                         po.tile([128, NT], F32, tag="o1", name=f"o1_{nt}")],
                        pz.tile([1, NT], F32, tag="z", name=f"z{nt}"),
                    )
                o_ps, _ = cur[nt]
                for j in range(GRP):
                    mb = g * GRP + j
                    esl = e_sb[:, j * NT:(j + 1) * NT]
                    first, last = mb == 0, mb == NMB - 1
                    for cc in range(2):
                        nc.tensor.matmul(
                            o_ps[cc],
                            v_sb[mb][:, cc * 128:(cc + 1) * 128],
                            esl,
                            start=first, stop=last,
                        )
                # softmax denominator: DVE tree-sums the 4 key blocks; the
                # ones-matmul itself is deferred one slot (see emit_z)
                e01 = esp.tile([128, NT], BF16, tag="e01", name=f"e01_{nt}_{g}")
                e23 = esp.tile([128, NT], BF16, tag="e23", name=f"e23_{nt}_{g}")
                es = esp.tile([128, NT], BF16, tag="es", name=f"es_{nt}_{g}")
                # the last group's es chain is on the tile's critical path
                # (its exp lands only one slot earlier): let it run ahead of
                # queued tail work in the DVE stream
                with tc.high_priority(offset=60 if g == NGRP - 1 else 0):
                    nc.vector.tensor_add(e01, e_sb[:, 0:NT], e_sb[:, NT:2 * NT])
                    nc.vector.tensor_add(e23, e_sb[:, 2 * NT:3 * NT], e_sb[:, 3 * NT:])
                    nc.vector.tensor_add(es, e01, e23)
                zq.append((nt, g, es))
                while len(zq) > 1:
                    emit_z(*zq.pop(0))
                if g == NGRP - 1:
                    # free O' immediately (z-independent): PSUM -> bf16 SBUF
                    o_ps, z_ps = cur.pop(nt)
                    osb = []
                    for cc in range(2):
                        ot = osbp.tile(
                            [128, NT], BF16, tag=f"os{cc}", name=f"os{cc}_{nt}"
                        )
                        nc.vector.tensor_copy(out=ot, in_=o_ps[cc])
                        osb.append(ot)
                    tails[nt] = {"osb": osb, "z_ps": z_ps}

            # Pipelined one group deep: T-pack(i) ... PV(i-1).  exp(i) on ACT
            # hides under PV(i-1) on PE.  Tail PE work is deferred: Z(i) lands
            # after PV(i+1); outproj halves land 3 and 4 slots after a tile's
            # last PV, so the PE never waits on the DVE/DMA tail chain.
            prev = None
            for nt in range(NTILES):
                nsl = slice(nt * NT, (nt + 1) * NT)
                for g in range(NGRP):
                    # 4-way row-packed score matmuls: strip j contracts its own
                    # 32 rows of the PE array concurrently (measured ~3x).
                    t_ps = pt.tile([128, GRP * NT], F32, tag="t", name=f"t{nt}_{g}")
                    for j in range(GRP):
                        nc.tensor.matmul(
                            t_ps[:, j * NT:(j + 1) * NT],
                            kt_stack[32 * j:32 * (j + 1), g, :],
                            qt_rep[32 * j:32 * (j + 1), nsl],
                            start=True, stop=True,
                            tile_position=(32 * j, 0),
                        )
                    e_sb = ep.tile([128, GRP * NT], BF16, tag="e", name=f"e{nt}_{g}")
                    nc.scalar.activation(out=e_sb, in_=t_ps, func=FT.Exp)
                    if prev is not None:
                        emit_oz(*prev)
                        if prev[0] >= 1:
                            pn = prev[0] - 1
                            if prev[1] == 2:
                                emit_tail_f(pn, 0)
                            elif prev[1] == 3:
                                emit_tail_f(pn, 1)
                            elif prev[1] == 4:
                                emit_tail_fn(pn, 0)
                            elif prev[1] == 5:
                                emit_tail_fn(pn, 1)
                    prev = (nt, g, e_sb)
            emit_oz(*prev)
            while zq:
                emit_z(*zq.pop(0))
            emit_tail_f(NTILES - 1, 0)
            emit_tail_f(NTILES - 1, 1)
            emit_tail_fn(NTILES - 1, 0)
            emit_tail_fn(NTILES - 1, 1)

    _split_multiwaits(nc)
    return nc


def _split_multiwaits(nc: bass.Bass) -> None:
    """This container's walrus accepts at most ONE sync-wait per instruction
    (CoreV3GenImpl setupSyncWait). Tile emits multi-wait instructions; split
    the excess waits onto EventSemaphore carriers inserted just before the
    instruction on the same engine — same-engine program order makes this
    semantics-preserving."""
    import json as _json

    data = _json.loads(mybir.module_to_json_bytes(nc.m))
    uid = 0
    for fn in data["functions"]:
        for bb in fn["blocks"]:
            new = []
            for inst in bb["instructions"]:
                si = inst.get("sync_info")
                waits = (si or {}).get("on_wait") or []
                if len(waits) > 1:
                    for wcmd in waits[:-1]:
                        uid += 1
                        new.append({
                            "debug": inst.get("debug", 0),
                            "engine": inst["engine"],
                            "ins": [], "outs": [],
                            "name": f"syncw-{uid}",
                            "opcode": "EventSemaphore",
                            "sync_info": {"on_update": [], "on_wait": [wcmd]},
                        })
                    si["on_wait"] = [waits[-1]]
                new.append(inst)
            bb["instructions"] = new
    nc.m = mybir.module_from_json_bytes(_json.dumps(data).encode())


_NC = None


def _get_nc():
    global _NC
    if _NC is None:
        _NC = build_nc()
    return _NC


def _prep_maps(x, Wf, bf, Wg, bg, Wh, bh, Wo, bo):
    bft = ml_dtypes.bfloat16
    # V bias folds through the (linear) attention average into the output
    # projection: out = (O'/z) @ Wo + (bh @ Wo + bo)
    bo_prime = (Wo.T.astype(np.float64) @ bh.astype(np.float64)
                + bo.astype(np.float64)).astype(np.float32)
    shared = {
        "wf": np.ascontiguousarray(Wf.reshape(2, 128, KEY).astype(bft)),
        "wg": np.ascontiguousarray(Wg.reshape(2, 128, KEY).astype(bft)),
        "wh": np.ascontiguousarray(Wh.reshape(2, 128, C).astype(bft)),
        "wo": np.ascontiguousarray(Wo.reshape(2, 128, C).astype(bft)),
        "bfp": np.ascontiguousarray(bf.reshape(KEY, 1).astype(np.float32)),
        "bgp": np.ascontiguousarray(bg.reshape(KEY, 1).astype(np.float32)),
        "bop": np.ascontiguousarray(bo_prime.reshape(2, 128, 1)),
    }
    in_maps = []
    for b in range(B):
        xTb = np.ascontiguousarray(
            x[b].reshape(N, C).T.astype(bft).reshape(2, 128, N)
        )
        m = dict(shared)
        m["xT"] = xTb
        in_maps.append(m)
    return in_maps


def run(x, Wf, bf, Wg, bg, Wh, bh, Wo, bo, trace=False, **kw):
    x = np.asarray(x, dtype=np.float32)
    in_maps = _prep_maps(
        x, *(np.asarray(a, dtype=np.float32) for a in (Wf, bf, Wg, bg, Wh, bh, Wo, bo))
    )
    res = run_bass_kernel_spmd(_get_nc(), in_maps, list(range(B)), trace=trace, **kw)
    out = np.empty((B, H, W, C), dtype=np.float32)
    for b in range(B):
        oT = np.asarray(res.results[b]["outT"], dtype=np.float32).reshape(C, N)
        out[b] = oT.T.reshape(H, W, C)
    return out, res


def kernel(x, Wf, bf, Wg, bg, Wh, bh, Wo, bo):
    out, _ = run(x, Wf, bf, Wg, bg, Wh, bh, Wo, bo)
    return out

